# revision 1
# baseline (speedup 1.0000x reference)
"""Bimamba (bidirectional Mamba) block on 8 trn2 NeuronCores.

Sharding: tensor-parallel over d_inner (256 channels/core). LayerNorm is
computed redundantly on every core; x_proj partial sums are AllReduced;
the out_proj contraction is resolved with an AllToAll that re-shards y
from (d_inner-shard, all tokens) to (all d_inner, token-shard).
"""
import sys, os, json, time

sys.path.insert(0, '/opt/trn_rl_repo')

import numpy as np
import concourse.bass as bass
import concourse.mybir as mybir
import concourse.tile as tile
import bass_rust
from concourse.vector_clock import ScopedClock
from concourse import bass2jax
import jax

# ----------------------------------------------------------------- patches

def _patched_drain_and_barrier(self, tick_clock, wait_clock):
    nc = self.nc
    gc = tick_clock.global_clock
    vals = json.loads(repr(gc).replace("VectorClock(", "").rstrip(")"))
    procs = [i for i, v in enumerate(vals) if v > 0]
    for p in procs:
        sub = bass_rust.VectorClock()
        sub.require_at_least(p, vals[p])
        nop = nc.sync.nop(nofuse=True)
        wait_clock.add_sem_waits(nop.ins, ScopedClock({None: sub}))
    nc.sync.drain()
    nc.all_engine_barrier()
    assert self.sems is not None
    popped = nc._tile_sem_poison_stack.pop()
    assert popped is self._sem_poison
    nc.clear_and_free_semaphores(list(self.sems.allocated().values()))
    nc.all_engine_barrier()


tile.TileContext._drain_and_barrier = _patched_drain_and_barrier

_SPLIT_ENGINES = {"SP", "PE", "DVE", "Activation", "Pool"}
_wsplit_ctr = [0]


def _split_excess_waits(bir, max_waits=1):
    for f in bir.get("functions") or []:
        for blk in f.get("blocks") or []:
            insts = blk.get("instructions") or []
            out = []
            for inst in insts:
                si = inst.get("sync_info")
                waits = (si or {}).get("on_wait") or []
                eng = inst.get("engine")
                if len(waits) > max_waits and eng in _SPLIT_ENGINES:
                    keep, extra = waits[:max_waits], waits[max_waits:]
                    for i in range(0, len(extra), max_waits):
                        _wsplit_ctr[0] += 1
                        out.append({
                            "debug": inst.get("debug", 0),
                            "engine": eng,
                            "ins": [], "outs": [],
                            "name": f"WSPLIT-{_wsplit_ctr[0]}",
                            "opcode": "NoOp",
                            "sync_info": {"on_update": [],
                                          "on_wait": extra[i:i + max_waits]},
                        })
                    si["on_wait"] = keep
                out.append(inst)
            blk["instructions"] = out
    return bir


if not getattr(bass.Bass, "_ws_patched", False):
    _orig_to_json_bytes = bass.Bass.to_json_bytes

    def _patched_to_json_bytes(self):
        bir = json.loads(_orig_to_json_bytes(self))
        _split_excess_waits(bir)
        return json.dumps(bir).encode()

    bass.Bass.to_json_bytes = _patched_to_json_bytes
    bass.Bass._ws_patched = True

# ----------------------------------------------------------------- consts

B, D, L = 2, 1024, 2048
DIN, NST, DTR, KCV = 2048, 16, 64, 4
NC_ = 8
DL = DIN // NC_          # 256 channels per core
TOK = B * L              # 4096 tokens, b-major
TSL = TOK // NC_         # 512-token output slice per core
EPS = 1e-5

f32 = mybir.dt.float32
f16 = mybir.dt.float16
AL = mybir.AluOpType
AF = mybir.ActivationFunctionType

SCAN_DT = f16            # dtype of scan operands (internal state is fp32)


# ----------------------------------------------------------------- program

def build_program(reps=1):
    nc = bass.Bass(trn_type="TRN2", target_bir_lowering=False, num_devices=NC_)

    def din(name, shape, dt=f32):
        return nc.dram_tensor(name, list(shape), dt, kind="ExternalInput").ap()

    def dout(name, shape, dt=f32):
        return nc.dram_tensor(name, list(shape), dt, kind="ExternalOutput").ap()

    hs_in = din("hs", (B, D, L))
    res_in = din("res", (B, D, L))
    wx_in = din("wxT", (D, DL), f16)       # in_proj x-rows lhsT (gamma folded)
    wz_in = din("wzT", (D, DL), f16)
    bx_in = din("bx", (DL, 1))             # in_proj beta-fold biases
    bz_in = din("bz", (DL, 1))
    cvd_in = din("convdiag", (2, KCV, 2, 128, 128), f16)   # (dir,tap,dt,.,.)
    cb_in = din("convb", (2, DL, 1))
    xw_in = din("xwT", (2, DL, DTR + 2 * NST), f16)        # (dir, k=dl, 96)
    dtw_in = din("dtwT", (2, DTR, DL), f16)
    dtb_in = din("dtb", (2, DL, 1))
    atab_in = din("atab", (2, DL, NST))
    dpd_in = din("dpdiag", (2, 2, 128, 128), f16)
    wop_in = din("wopT", (DIN, D), f16)
    opb_in = din("opb", (D, 1))
    i128_in = din("i128", (128, 128), f16)
    ones_in = din("ones", (128, 1), f16)

    r_out = dout("r_out", (D, TOK))        # full r, b-major columns
    o_out = dout("o_out", (D, TSL))        # out token-slice (per core)
    DBG = os.environ.get("KERNEL_DEBUG") == "1"
    ABL = set((os.environ.get("KERNEL_ABLATE") or "").split(","))
    if DBG:
        d_xz = dout("d_xz", (128, L + 6), f16)
        d_zs = dout("d_zs", (128, TOK), f16)
        d_u00 = dout("d_u00", (128, TOK), f16)
        d_u10 = dout("d_u10", (128, TOK), f16)
        d_xdbl = dout("d_xdbl", (192, TOK))
        d_dt00 = dout("d_dt00", (128, TOK), f16)
        d_du00 = dout("d_du00", (128, TOK), f16)
        d_y0 = dout("d_y0", (128, TOK), f16)
        d_yall = dout("d_yall", (DIN, TSL), f16)

    with tile.TileContext(nc) as tc:
        with tc.tile_pool(name="wts", bufs=1) as wts, \
             tc.tile_pool(name="dram", bufs=1, space="DRAM") as dram:

            # ---- load small weights
            wx_sb = [wts.tile([128, DL], f16, tag=f"wx{k}", name=f"wx{k}") for k in range(8)]
            wz_sb = [wts.tile([128, DL], f16, tag=f"wz{k}", name=f"wz{k}") for k in range(8)]
            for k in range(8):
                nc.sync.dma_start(wx_sb[k][:], wx_in[k * 128:(k + 1) * 128, :])
                nc.sync.dma_start(wz_sb[k][:], wz_in[k * 128:(k + 1) * 128, :])
            bx_sb = [wts.tile([128, 1], f32, tag=f"bx{m}", name=f"bx{m}") for m in range(2)]
            bz_sb = [wts.tile([128, 1], f32, tag=f"bz{m}", name=f"bz{m}") for m in range(2)]
            for m in range(2):
                nc.sync.dma_start(bx_sb[m][:], bx_in[m * 128:(m + 1) * 128, :])
                nc.sync.dma_start(bz_sb[m][:], bz_in[m * 128:(m + 1) * 128, :])
            cvd_sb = {}
            for dr in range(2):
                for j in range(KCV):
                    for m in range(2):
                        t = wts.tile([128, 128], f16, tag=f"cv{dr}{j}{m}", name=f"cv{dr}{j}{m}")
                        nc.sync.dma_start(t[:], cvd_in[dr, j, m])
                        cvd_sb[dr, j, m] = t
            cb_sb = {}
            dtb_sb = {}
            at_sb = {}
            dpd_sb = {}
            for dr in range(2):
                for m in range(2):
                    t = wts.tile([128, 1], f32, tag=f"cb{dr}{m}", name=f"cb{dr}{m}")
                    nc.sync.dma_start(t[:], cb_in[dr, m * 128:(m + 1) * 128, :])
                    cb_sb[dr, m] = t
                    t = wts.tile([128, 1], f32, tag=f"db{dr}{m}", name=f"db{dr}{m}")
                    nc.sync.dma_start(t[:], dtb_in[dr, m * 128:(m + 1) * 128, :])
                    dtb_sb[dr, m] = t
                    t = wts.tile([128, NST], f32, tag=f"at{dr}{m}", name=f"at{dr}{m}")
                    nc.sync.dma_start(t[:], atab_in[dr, m * 128:(m + 1) * 128, :])
                    at_sb[dr, m] = t
                    t = wts.tile([128, 128], f16, tag=f"dp{dr}{m}", name=f"dp{dr}{m}")
                    nc.sync.dma_start(t[:], dpd_in[dr, m])
                    dpd_sb[dr, m] = t
            xw_sb = {}
            for dr in range(2):
                for m in range(2):
                    t = wts.tile([128, DTR + 2 * NST], f16, tag=f"xw{dr}{m}", name=f"xw{dr}{m}")
                    nc.sync.dma_start(t[:], xw_in[dr, m * 128:(m + 1) * 128, :])
                    xw_sb[dr, m] = t
            dtw_sb = {}
            for dr in range(2):
                t = wts.tile([DTR, DL], f16, tag=f"dtw{dr}", name=f"dtw{dr}")
                nc.sync.dma_start(t[:], dtw_in[dr])
                dtw_sb[dr] = t
            i128_sb = wts.tile([128, 128], f16, tag="i128", name="i128")
            nc.sync.dma_start(i128_sb[:], i128_in)
            ones_sb = wts.tile([128, 1], f16, tag="ones", name="ones")
            nc.sync.dma_start(ones_sb[:], ones_in)
            opb_sb = [wts.tile([128, 1], f32, tag=f"opb{m}", name=f"opb{m}") for m in range(8)]
            for m in range(8):
                nc.sync.dma_start(opb_sb[m][:], opb_in[m * 128:(m + 1) * 128, :])

            for _rep in range(reps):
                # ---- explicit-lifetime activation pools (stack order!)
                cm_zs = tc.tile_pool(name="zsp", bufs=1)
                zsp = cm_zs.__enter__()
                zs16 = [zsp.tile([128, TOK], f16, tag=f"zs{m}", name=f"zs{m}") for m in range(2)]
                cm_u = tc.tile_pool(name="up", bufs=1)
                upool = cm_u.__enter__()
                u16 = {(dr, m): upool.tile([128, TOK], f16, tag=f"u{dr}{m}", name=f"u{dr}{m}")
                       for dr in range(2) for m in range(2)}
                cm_xp = tc.tile_pool(name="xpp", bufs=1)
                xpp = cm_xp.__enter__()
                xpad = {(m, b): xpp.tile([128, L + 6], f16, tag=f"xp{m}{b}", name=f"xp{m}{b}")
                        for m in range(2) for b in range(2)}
                for m in range(2):
                    for b in range(2):
                        nc.vector.memset(xpad[m, b][:, 0:3], 0.0)
                        nc.vector.memset(xpad[m, b][:, L + 3:L + 6], 0.0)

                # ========== Phases A+B per batch: LN + in_proj ================
                cm_rhn = tc.tile_pool(name="rhn", bufs=1)
                rhn = cm_rhn.__enter__()
                with tc.tile_pool(name="lnw", bufs=1) as lnw, \
                     tc.tile_pool(name="lnps", bufs=1, space="PSUM") as lnps, \
                     tc.tile_pool(name="lnsm", bufs=1) as lnsm, \
                     tc.tile_pool(name="bps", bufs=4, space="PSUM") as bps:
                    HW = 1024
                    for b in range(2):
                        hn = [rhn.tile([128, L], f16, tag=f"hn{k}", name=f"hn{k}")
                              for k in range(8)]
                        sst = [lnps.tile([33, 512], f32, tag=f"ss{c}", name=f"ss{c}") for c in range(4)]
                        ssum = [t[0:1, :] for t in sst]
                        ssq = [t[32:33, :] for t in sst]
                        for k in range(8):
                            for h in range(2):
                                hsl = slice(h * HW, (h + 1) * HW)
                                hs_t = lnw.tile([128, HW], f32, tag="hs", name="hs_t", bufs=2)
                                re_t = lnw.tile([128, HW], f32, tag="re", name="re_t", bufs=2)
                                nc.sync.dma_start(hs_t[:], hs_in[b, k * 128:(k + 1) * 128, hsl])
                                nc.sync.dma_start(re_t[:], res_in[b, k * 128:(k + 1) * 128, hsl])
                                nc.vector.tensor_tensor(hs_t[:], hs_t[:], re_t[:], AL.add)
                                nc.sync.dma_start(
                                    r_out[k * 128:(k + 1) * 128, b * L + h * HW:b * L + (h + 1) * HW],
                                    hs_t[:])
                                rs = hn[k][:, hsl]
                                nc.vector.tensor_copy(rs, hs_t[:])
                                sq_t = lnw.tile([128, HW], f16, tag="sqt", name="sq_t", bufs=2)
                                nc.scalar.activation(sq_t[:], rs, AF.Square)
                                for c in range(2):
                                    cc = h * 2 + c
                                    nc.tensor.matmul(ssum[cc], ones_sb[:],
                                                     rs[:, c * 512:(c + 1) * 512],
                                                     start=(k == 0), stop=(k == 7))
                                    nc.tensor.matmul(ssq[cc], ones_sb[:],
                                                     sq_t[:, c * 512:(c + 1) * 512],
                                                     start=(k == 0), stop=(k == 7))
                        mu = lnsm.tile([1, L], f32, tag="mu", name="mu")
                        ex2 = lnsm.tile([1, L], f32, tag="ex2", name="ex2")
                        for c in range(4):
                            nc.vector.tensor_scalar_mul(
                                mu[:, c * 512:(c + 1) * 512], ssum[c], 1.0 / D)
                            nc.vector.tensor_scalar_mul(
                                ex2[:, c * 512:(c + 1) * 512], ssq[c], 1.0 / D)
                        tmp = lnsm.tile([1, L], f32, tag="tmp", name="tmp")
                        nc.vector.tensor_tensor(tmp[:], mu[:], mu[:], AL.mult)
                        nc.vector.tensor_tensor(ex2[:], ex2[:], tmp[:], AL.subtract)
                        nc.vector.tensor_scalar_add(ex2[:], ex2[:], float(EPS))
                        nc.scalar.activation(ex2[:], ex2[:], AF.Sqrt)
                        nc.vector.reciprocal(tmp[:], ex2[:])
                        r16_ = lnsm.tile([1, L], f16, tag="r16_", name="r16_")
                        m16_ = lnsm.tile([1, L], f16, tag="m16_", name="m16_")
                        nc.vector.tensor_copy(r16_[:], tmp[:])
                        nc.vector.tensor_copy(m16_[:], mu[:])
                        drow = dram.tile([2, L], f16, tag="stat", name="stat")
                        nc.sync.dma_start(drow[0:1, :], r16_[:])
                        nc.sync.dma_start(drow[1:2, :], m16_[:])
                        rb = lnsm.tile([128, L], f16, tag="rbc", name="rbc")
                        mb = lnsm.tile([128, L], f16, tag="mbc", name="mbc")
                        nc.sync.dma_start(rb[:], drow[0:1, :].broadcast_to((128, L)))
                        nc.sync.dma_start(mb[:], drow[1:2, :].broadcast_to((128, L)))
                        # hn = (r - mu) * rstd, in place
                        for k in range(8):
                            nc.vector.tensor_tensor(hn[k][:], hn[k][:], mb[:], AL.subtract)
                            nc.vector.tensor_tensor(hn[k][:], hn[k][:], rb[:], AL.mult)
                        # ---- in_proj for this batch
                        for m in range(4):      # 0,1 = x halves; 2,3 = z halves
                            for ch in range(4):
                                ps = bps.tile([128, 512], f32, tag="ps", name="ps")
                                for k in range(8):
                                    w = wx_sb[k] if m < 2 else wz_sb[k]
                                    lh = w[:, (m % 2) * 128:(m % 2) * 128 + 128]
                                    nc.tensor.matmul(ps[:], lh,
                                                     hn[k][:, ch * 512:(ch + 1) * 512],
                                                     start=(k == 0), stop=(k == 7))
                                col = ch * 512
                                if m < 2:
                                    dst = xpad[m, b][:, 3 + col:3 + col + 512]
                                    nc.scalar.activation(dst, ps[:], AF.Identity,
                                                         bias=bx_sb[m][:])
                                else:
                                    dst = zs16[m - 2][:, b * L + col:b * L + col + 512]
                                    nc.scalar.activation(dst, ps[:], AF.Silu,
                                                         bias=bz_sb[m - 2][:])
                cm_rhn.__exit__(None, None, None)   # free hn

                # ================= Phase C: causal conv + silu ===============
                with tc.tile_pool(name="cps", bufs=4, space="PSUM") as cps, \
                     tc.tile_pool(name="xrv", bufs=1) as xrv:
                    xrev = {}
                    for m in range(2):
                        for b in range(2):
                            t = xrv.tile([128, L + 6], f16, tag=f"xr{m}{b}", name=f"xr{m}{b}")
                            nc.vector.tensor_copy(t[:], xpad[m, b][:, L + 5::-1])
                            xrev[m, b] = t
                    for dr in range(2):
                        for m in range(2):
                            for b in range(2):
                                src_t = xpad[m, b] if dr == 0 else xrev[m, b]
                                for c in range(4):
                                    ps = cps.tile([128, 512], f32, tag="ps", name="ps")
                                    for j in range(KCV):
                                        rhs = src_t[:, j + c * 512:j + c * 512 + 512]
                                        nc.tensor.matmul(ps[:], cvd_sb[dr, j, m], rhs,
                                                         start=(j == 0), stop=(j == KCV - 1))
                                    dst = u16[dr, m][:, b * L + c * 512:b * L + (c + 1) * 512]
                                    nc.scalar.activation(dst, ps[:], AF.Silu,
                                                         bias=cb_sb[dr, m][:])
                cm_xp.__exit__(None, None, None)    # free xpad

                # ================= Phase D: x_proj + AllReduce + dt ==========
                cm_dt = tc.tile_pool(name="dtp", bufs=1)
                dtpool = cm_dt.__enter__()
                dt16 = {(dr, m): dtpool.tile([128, TOK], f16, tag=f"dt{dr}{m}", name=f"dt{dr}{m}")
                        for dr in range(2) for m in range(2)}
                dtu16 = {(dr, m): dtpool.tile([128, TOK], f16, tag=f"du{dr}{m}", name=f"du{dr}{m}")
                         for dr in range(2) for m in range(2)}
                y16 = [dtpool.tile([128, TOK], f16, tag=f"y{m}", name=f"y{m}") for m in range(2)]
                NXP = DTR + 2 * NST   # 96
                ar_src = dram.tile([2 * NXP, TOK], f32, tag="arsrc", name="arsrc")
                ar_dst = dram.tile([2 * NXP, TOK], f32, tag="ardst", name="ardst",
                                   addr_space="Shared")
                with tc.tile_pool(name="dps", bufs=4, space="PSUM") as dps, \
                     tc.tile_pool(name="dwk", bufs=1) as dwk:
                    for dr in range(2):
                        for ch in range(8):
                            ps = dps.tile([NXP, 512], f32, tag="ps", name="ps")
                            for m in range(2):
                                nc.tensor.matmul(ps[:], xw_sb[dr, m],
                                                 u16[dr, m][:, ch * 512:(ch + 1) * 512],
                                                 start=(m == 0), stop=(m == 1))
                            xc = dwk.tile([NXP, 512], f32, tag="xc", name="xc", bufs=3)
                            nc.scalar.activation(xc[:], ps[:], AF.Identity)
                            nc.sync.dma_start(
                                ar_src[dr * NXP:(dr + 1) * NXP, ch * 512:(ch + 1) * 512],
                                xc[:])
                    nc.gpsimd.collective_compute(
                        "AllReduce", AL.add, replica_groups=[list(range(NC_))],
                        ins=[ar_src.opt()], outs=[ar_dst.opt()])

                    bc_src = dram.tile([2 * 2 * NST, TOK], f16, tag="bcsrc", name="bcsrc")
                    for dr in range(2):
                        dtp16 = dwk.tile([DTR, TOK], f16, tag="dtp16", name="dtp16")
                        for h in range(2):
                            hsl = slice(h * L, (h + 1) * L)
                            dtp = dwk.tile([DTR, L], f32, tag="dtp", name="dtp")
                            nc.sync.dma_start(dtp[:], ar_dst[dr * NXP:dr * NXP + DTR, hsl])
                            nc.vector.tensor_copy(dtp16[:, hsl], dtp[:])
                            bcf = dwk.tile([2 * NST, L], f32, tag="bcf", name="bcf")
                            nc.sync.dma_start(bcf[:], ar_dst[dr * NXP + DTR:(dr + 1) * NXP, hsl])
                            bch = dwk.tile([2 * NST, L], f16, tag="bch", name="bch")
                            nc.vector.tensor_copy(bch[:], bcf[:])
                            nc.sync.dma_start(bc_src[dr * 2 * NST:(dr + 1) * 2 * NST, hsl], bch[:])
                        # dt = softplus(dtw @ dtpart + dtb) via Exp then Ln(x+1)
                        for m in range(2):
                            for ch in range(8):
                                ps = dps.tile([128, 512], f32, tag="ps2", name="ps2")
                                nc.tensor.matmul(ps[:],
                                                 dtw_sb[dr][:, m * 128:(m + 1) * 128],
                                                 dtp16[:, ch * 512:(ch + 1) * 512],
                                                 start=True, stop=True)
                                et = dwk.tile([128, 512], f32, tag="et", name="et", bufs=3)
                                nc.scalar.activation(et[:], ps[:], AF.Exp,
                                                     bias=dtb_sb[dr, m][:])
                                nc.scalar.activation(
                                    dt16[dr, m][:, ch * 512:(ch + 1) * 512], et[:],
                                    AF.Ln, bias=1.0)
                    for dr in range(2):
                        for m in range(2):
                            nc.vector.tensor_tensor(dtu16[dr, m][:], dt16[dr, m][:],
                                                    u16[dr, m][:], AL.mult)

                # ================= Phase E: selective scan ===================
                for dr in range(2):
                    for b in range(2):
                        bsl = slice(b * L, (b + 1) * L)
                        with tc.tile_pool(name=f"eps{dr}{b}", bufs=1, space="PSUM") as eps, \
                             tc.tile_pool(name=f"esw{dr}{b}", bufs=2) as esw:
                            py = {(m, c): eps.tile([128, 512], f32, tag=f"py{m}{c}", name=f"py{m}{c}")
                                  for m in range(2) for c in range(4)}
                            h16s = {}
                            for n in range(NST):
                                if "nobc" not in ABL:
                                    bt = esw.tile([128, L], f16, tag="bt", name="bt")
                                    nc.sync.dma_start(
                                        bt[:], bc_src[dr * 2 * NST + n:dr * 2 * NST + n + 1,
                                                      bsl].broadcast_to((128, L)))
                                    ct = esw.tile([128, L], f16, tag="ct", name="ct")
                                    nc.sync.dma_start(
                                        ct[:], bc_src[dr * 2 * NST + NST + n:
                                                      dr * 2 * NST + NST + n + 1,
                                                      bsl].broadcast_to((128, L)))
                                for m in range(2):
                                    h16 = esw.tile([128, L], SCAN_DT, tag="h16", name="h16")
                                    if "noscan" not in ABL:
                                        a16 = esw.tile([128, L], SCAN_DT, tag="a16", name="a16")
                                        nc.scalar.activation(a16[:], dt16[dr, m][:, bsl],
                                                             AF.Exp,
                                                             scale=at_sb[dr, m][:, n:n + 1])
                                        xs = esw.tile([128, L], SCAN_DT, tag="xs", name="xs")
                                        nc.vector.tensor_tensor(xs[:], dtu16[dr, m][:, bsl],
                                                                bt[:], AL.mult)
                                        nc.vector.tensor_tensor_scan(h16[:], a16[:], xs[:],
                                                                     0.0, AL.mult, AL.add)
                                        nc.vector.tensor_tensor(h16[:], h16[:], ct[:], AL.mult)
                                    if "nonsum" not in ABL:
                                        for c in range(4):
                                            nc.tensor.matmul(py[m, c][:], i128_sb[:],
                                                             h16[:, c * 512:(c + 1) * 512],
                                                             start=(n == 0), stop=False)
                                    elif n == 0:
                                        for c in range(4):
                                            nc.tensor.matmul(py[m, c][:], i128_sb[:],
                                                             h16[:, c * 512:(c + 1) * 512],
                                                             start=True, stop=False)
                            for m in range(2):
                                for c in range(4):
                                    nc.tensor.matmul(
                                        py[m, c][:], dpd_sb[dr, m],
                                        u16[dr, m][:, b * L + c * 512:b * L + (c + 1) * 512],
                                        start=False, stop=True)
                            for m in range(2):
                                for c in range(4):
                                    csl = slice(b * L + c * 512, b * L + (c + 1) * 512)
                                    if dr == 0:
                                        nc.vector.tensor_tensor(y16[m][:, csl], py[m, c][:],
                                                                zs16[m][:, csl], AL.mult)
                                    else:
                                        gt = esw.tile([128, 512], f16, tag="gt", name="gt")
                                        rc = 3 - c
                                        rev = py[m, rc][:, 511::-1]
                                        nc.vector.tensor_tensor(gt[:], rev,
                                                                zs16[m][:, csl], AL.mult)
                                        nc.vector.tensor_tensor(y16[m][:, csl],
                                                                y16[m][:, csl], gt[:], AL.add)

                if DBG:
                    nc.sync.dma_start(d_xz, xpad[0, 0][:])
                    nc.sync.dma_start(d_zs, zs16[0][:])
                    nc.sync.dma_start(d_u00, u16[0, 0][:])
                    nc.sync.dma_start(d_u10, u16[1, 0][:])
                    nc.sync.dma_start(d_xdbl, ar_dst[:])
                    nc.sync.dma_start(d_dt00, dt16[0, 0][:])
                    nc.sync.dma_start(d_du00, dtu16[0, 0][:])
                    nc.sync.dma_start(d_y0, y16[0][:])

                # ================= Phase F: AllToAll re-shard ================
                y_src = dram.tile([DIN, TSL], f16, tag="ysrc", name="ysrc")
                y_dst = dram.tile([DIN, TSL], f16, tag="ydst", name="ydst")
                for j in range(NC_):
                    for m in range(2):
                        nc.sync.dma_start(
                            y_src[j * DL + m * 128:j * DL + (m + 1) * 128, :],
                            y16[m][:, j * TSL:(j + 1) * TSL])
                nc.gpsimd.collective_compute(
                    "AllToAll", AL.bypass, replica_groups=[list(range(NC_))],
                    ins=[y_src.opt()], outs=[y_dst.opt()])
                if DBG:
                    nc.sync.dma_start(d_yall, y_dst[:])
                cm_dt.__exit__(None, None, None)
                cm_u.__exit__(None, None, None)
                cm_zs.__exit__(None, None, None)

                # ================= Phase G: out_proj =========================
                with tc.tile_pool(name="gw", bufs=1) as gw, \
                     tc.tile_pool(name="gps", bufs=4, space="PSUM") as gps, \
                     tc.tile_pool(name="gwk", bufs=3) as gwk:
                    yall = [gw.tile([128, TSL], f16, tag=f"ya{k}", name=f"ya{k}") for k in range(16)]
                    for k in range(16):
                        nc.sync.dma_start(yall[k][:], y_dst[k * 128:(k + 1) * 128, :])
                    wop_sb = [gw.tile([128, D], f16, tag=f"wo{k}", name=f"wo{k}") for k in range(16)]
                    for k in range(16):
                        nc.sync.dma_start(wop_sb[k][:], wop_in[k * 128:(k + 1) * 128, :])
                    for mt in range(8):
                        ps = gps.tile([128, TSL], f32, tag="ps", name="ps")
                        for k in range(16):
                            nc.tensor.matmul(ps[:], wop_sb[k][:, mt * 128:(mt + 1) * 128],
                                             yall[k][:], start=(k == 0), stop=(k == 15))
                        o32 = gwk.tile([128, TSL], f32, tag="o32", name="o32")
                        nc.scalar.activation(o32[:], ps[:], AF.Identity,
                                             bias=opb_sb[mt][:])
                        nc.sync.dma_start(o_out[mt * 128:(mt + 1) * 128, :], o32[:])
    return nc


# ----------------------------------------------------------------- host

def _host_prep(inputs):
    """Build per-core input dicts from the full-model inputs."""
    gam = np.asarray(inputs["gamma"], np.float32)
    bet = np.asarray(inputs["beta"], np.float32)
    wip = np.asarray(inputs["in_proj_w"], np.float32)     # (2*DIN, D)
    wop = np.asarray(inputs["out_proj_w"], np.float32)    # (D, DIN)
    opb = np.asarray(inputs["out_proj_b"], np.float32)
    hs = np.asarray(inputs["hidden_states"], np.float32)
    res = np.asarray(inputs["residual"], np.float32)

    conv_w = [np.asarray(inputs["conv_w"], np.float32),
              np.asarray(inputs["conv_w_b"], np.float32)]
    conv_b = [np.asarray(inputs["conv_b"], np.float32),
              np.asarray(inputs["conv_b_b"], np.float32)]
    xw = [np.asarray(inputs["xproj_w"], np.float32),
          np.asarray(inputs["xproj_w_b"], np.float32)]
    dtw = [np.asarray(inputs["dtproj_w"], np.float32),
           np.asarray(inputs["dtproj_w_b"], np.float32)]
    dtb = [np.asarray(inputs["dtproj_b"], np.float32),
           np.asarray(inputs["dtproj_b_b"], np.float32)]
    alog = [np.asarray(inputs["A_log"], np.float32),
            np.asarray(inputs["A_b_log"], np.float32)]
    dp = [np.asarray(inputs["Dp"], np.float32),
          np.asarray(inputs["Dp_b"], np.float32)]

    wip_g = wip * gam[None, :]           # fold gamma
    bias_full = wip @ bet                # fold beta  (2*DIN,)

    i128 = np.eye(128, dtype=np.float16)
    ones = np.ones((128, 1), np.float16)

    in_maps = []
    for i in range(NC_):
        ds = slice(i * DL, (i + 1) * DL)
        wxT = wip_g[ds, :].T.astype(np.float16)               # (D, DL)
        wzT = wip_g[DIN + i * DL:DIN + (i + 1) * DL, :].T.astype(np.float16)
        bx = bias_full[ds].reshape(DL, 1).astype(np.float32)
        bz = bias_full[DIN + i * DL:DIN + (i + 1) * DL].reshape(DL, 1).astype(np.float32)
        cvd = np.zeros((2, KCV, 2, 128, 128), np.float16)
        cb = np.zeros((2, DL, 1), np.float32)
        xwT = np.zeros((2, DL, DTR + 2 * NST), np.float16)
        dtwT = np.zeros((2, DTR, DL), np.float16)
        dtbv = np.zeros((2, DL, 1), np.float32)
        atab = np.zeros((2, DL, NST), np.float32)
        dpd = np.zeros((2, 2, 128, 128), np.float16)
        for dr in range(2):
            w = conv_w[dr][ds, 0, :]                          # (DL, KCV)
            for j in range(KCV):
                for m in range(2):
                    cvd[dr, j, m] = np.diag(w[m * 128:(m + 1) * 128, j]).astype(np.float16)
            cb[dr] = conv_b[dr][ds].reshape(DL, 1)
            xwT[dr] = xw[dr][:, ds].T.astype(np.float16)      # (DL, 96)
            dtwT[dr] = dtw[dr][ds, :].T.astype(np.float16)    # (DTR, DL)
            dtbv[dr] = dtb[dr][ds].reshape(DL, 1)
            atab[dr] = -np.exp(alog[dr][ds, :])
            for m in range(2):
                dpd[dr, m] = np.diag(dp[dr][ds][m * 128:(m + 1) * 128]).astype(np.float16)
        in_maps.append({
            "hs": hs, "res": res,
            "wxT": wxT, "wzT": wzT, "bx": bx, "bz": bz,
            "convdiag": cvd, "convb": cb,
            "xwT": xwT, "dtwT": dtwT, "dtb": dtbv, "atab": atab,
            "dpdiag": dpd,
            "wopT": wop.T.astype(np.float16),                 # (DIN, D)
            "opb": opb.reshape(D, 1).astype(np.float32),
            "i128": i128, "ones": ones,
        })
    return in_maps


class _Exec:
    """Compile once; run via PJRT shard_map on 8 cores."""

    def __init__(self, nc, n_cores):
        from jax.sharding import Mesh, PartitionSpec
        from jax.experimental.shard_map import shard_map
        bass2jax.install_neuronx_cc_hook()
        self.nc = nc
        self.n = n_cores
        partition_name = nc.partition_id_tensor.name if nc.partition_id_tensor else None
        in_names, out_names, out_avals, zero_outs = [], [], [], []
        for alloc in nc.m.functions[0].allocations:
            if not isinstance(alloc, mybir.MemoryLocationSet):
                continue
            name = alloc.memorylocations[0].name
            if alloc.kind == "ExternalInput":
                if name != partition_name:
                    in_names.append(name)
            elif alloc.kind == "ExternalOutput":
                shape = tuple(alloc.tensor_shape)
                npdt = mybir.dt.np(alloc.dtype)
                out_names.append(name)
                out_avals.append(jax.core.ShapedArray(shape, npdt))
                zero_outs.append(np.zeros(shape, npdt))
        self.in_names, self.out_names = in_names, out_names
        self.out_avals, self.zero_outs = out_avals, zero_outs
        all_in = list(in_names) + list(out_names)
        if partition_name is not None:
            all_in.append(partition_name)

        def _body(*args):
            operands = list(args)
            if partition_name is not None:
                operands.append(bass2jax.partition_id_tensor())
            outs = bass2jax._bass_exec_p.bind(
                *operands,
                out_avals=tuple(out_avals),
                in_names=tuple(all_in),
                out_names=tuple(out_names),
                lowering_input_output_aliases=(),
                sim_require_finite=True,
                sim_require_nnan=True,
                nc=nc,
            )
            return tuple(outs)

        devices = jax.devices()[:n_cores]
        self.mesh = Mesh(np.asarray(devices), ("core",))
        np_ = len(in_names) + len(out_names)
        self.fn = jax.jit(
            shard_map(_body, mesh=self.mesh,
                      in_specs=(PartitionSpec("core"),) * np_,
                      out_specs=(PartitionSpec("core"),) * len(out_names),
                      check_rep=False),
            keep_unused=True)

    def prep(self, in_maps):
        from jax.sharding import NamedSharding, PartitionSpec
        n = self.n
        cat = [np.concatenate([np.asarray(in_maps[c][k]) for c in range(n)], axis=0)
               for k in self.in_names]
        cat += [np.zeros((n * z.shape[0], *z.shape[1:]), z.dtype)
                for z in self.zero_outs]
        sh = NamedSharding(self.mesh, PartitionSpec("core"))
        return [jax.device_put(a, sh) for a in cat]

    def run(self, args):
        outs = self.fn(*args)
        jax.block_until_ready(outs)
        return outs

    def results(self, outs):
        n = self.n
        return [
            {name: np.asarray(outs[i]).reshape(n, *self.out_avals[i].shape)[c]
             for i, name in enumerate(self.out_names)}
            for c in range(n)
        ]


_EXEC = None


def _get_exec():
    global _EXEC
    if _EXEC is None:
        _EXEC = _Exec(build_program(), NC_)
    return _EXEC


def kernel(**inputs):
    e = _get_exec()
    in_maps = _host_prep(inputs)
    res = e.results(e.run(e.prep(in_maps)))
    # r: (D, TOK) b-major -> (B, D, L)
    r = res[0]["r_out"].reshape(D, B, L).transpose(1, 0, 2).astype(np.float32)
    out = np.zeros((B, D, L), np.float32)
    for i in range(NC_):
        b = i // 4
        l0 = (i % 4) * TSL
        out[b][:, l0:l0 + TSL] = res[i]["o_out"]
    return out, r

